# revision 28
# baseline (speedup 1.0000x reference)
"""Trainium2 Bass kernel for a GNN message-passing layer.

Reference semantics (per edge e = (src j, dst i)):
    m_in  = [x_j, pos_j - pos_i]                 # [E, 6]
    h     = celu(m_in @ f_w1 + f_b1)             # [E, 64]
    msg   = relu(h @ f_w2 + f_b2)                # [E, 64]
    aggr  = segment_max(msg, dst, N); empty -> 0 # [N, 64]
    u     = celu([aggr, x] @ g_w1 + g_b1)
    out   = celu(u @ g_w2 + g_b2)                # [N, 64]

Sharding: nodes split into 8 contiguous ranges (6250/core); each core gets the
edges whose dst is in its range, so segment-max is local.  Host does
index-only work (degree-sort, round layout, gather); device does every FLOP.

Device program (v2): celu decomposed as celu(z) = relu(-z) + exp(-relu(-z))
+ z - 1.  Per 1024-column group (2 edges stacked per column):
  zb = w9@f (PSUM), then either
    A-path: r = ACT.Relu(-zb-b1), e = ACT.Exp(-r); ms += w2@r + w2@e
    D-path: m = DVE.ts(zb+b1 min 0) (= -r), e = ACT.Exp(m); ms += (-w2)@m + w2@e
  ms also accumulates w12@f (the linear z term), then DVE tensor-max into a
  bf16 running aggregate (relu+bias deferred past the max).
The PE stream is software-pipelined depth-2 (w2-streams of group g run while
zb of g+2 and ms-init of g+1 are computed) so the tensor engine never waits
on ACT; a gap-free warmup burst un-throttles the PE HAM clock gate
(1.2 -> 2.4 GHz) at kernel start and keep-warm dummies span the node-phase
lead-in.
"""

import math
import os
import sys

import numpy as np

N = 50000
E = 1600000
CORES = 8
NCN = N // CORES            # nodes per core
TILE = 512                  # fp32 matmul moving free dim / one PSUM bank
GRP = 1024                  # group width (columns) = 2 tiles
SUP = 4096                  # feats DMA staging superblock (columns) = 4 groups
F32 = np.float32
DPAT = 3                    # every DPAT-th group takes the DVE (m) path


# --------------------------------------------------------------------------
# host-side layout (index work only)
# --------------------------------------------------------------------------

def _core_layouts(edge_index):
    """Per-core node ordering + degree-sorted CSR of local edges."""
    dst = np.asarray(edge_index[1])
    cores = []
    for c in range(CORES):
        lo, hi = c * NCN, (c + 1) * NCN
        eids = np.nonzero((dst >= lo) & (dst < hi))[0]
        ldst = (dst[eids] - lo).astype(np.int64)
        deg = np.bincount(ldst, minlength=NCN)
        order = np.argsort(-deg, kind="stable")         # node ranks
        rank = np.empty(NCN, np.int64)
        rank[order] = np.arange(NCN)
        perm = np.argsort(rank[ldst], kind="stable")
        es = eids[perm]                                  # edges sorted by rank
        deg_s = deg[order]
        row_start = np.zeros(NCN + 1, np.int64)
        np.cumsum(deg_s, out=row_start[1:])
        cores.append(dict(es=es, deg_s=deg_s, row_start=row_start,
                          order=order, empty=order[deg_s == 0] + lo))
    return cores


def _tile_plan(cores):
    """Shared (SPMD-uniform) tile plan at 512-column granularity.

    tiles: list of (pair_round t, node_block k); tile covers node ranks
    [512k, 512k+512) at rounds (2t, 2t+1).  Flat consecutive pairs of tiles
    form 1024-column groups (groups may straddle rounds; the aggregate-max
    is per-tile anyway).
    """
    rmax = max(int(c["deg_s"][0]) for c in cores)
    n_pairs = (rmax + 1) // 2
    tiles = []
    for t in range(n_pairs):
        w = max(int(np.searchsorted(-c["deg_s"], -(2 * t), side="left"))
                for c in cores)      # max over cores of #nodes with deg > 2t
        if t == 0:
            w = NCN                  # every aggr column gets initialized
        for k in range(max(1, (w + TILE - 1) // TILE)):
            tiles.append((t, k))
    if len(tiles) % 2:
        assert tiles[-1][0] > 0
        tiles.append(tiles[-1])      # dup: max is idempotent, not first-touch
    S = TILE * len(tiles)
    ncw = TILE * ((NCN + TILE - 1) // TILE)
    return tiles, S, ncw


def _pack_core(core, tiles, S, ncw, x, pos, src, dst):
    """Build one core's slot->edge assignment and gather features."""
    es, deg_s, row_start = core["es"], core["deg_s"], core["row_start"]
    ncols = len(tiles) * TILE
    nvec = np.tile(np.arange(TILE, dtype=np.int64), len(tiles))  # col in tile
    kvec = np.repeat([k for (_, k) in tiles], TILE)
    tvec = np.repeat([t for (t, _) in tiles], TILE)
    node = kvec * TILE + nvec                    # node rank targeted by column

    safe_node = np.minimum(node, NCN - 1)
    ecap = len(es) - 1
    first_edge = es[np.minimum(row_start[safe_node], ecap)]  # dup fallback
    bad = (node >= NCN) | (deg_s[safe_node] == 0)
    first_edge = np.where(bad, es[0], first_edge)

    def round_edges(r):
        has = (~bad) & (deg_s[safe_node] > r)
        idx = np.minimum(row_start[safe_node] + np.where(has, r, 0), ecap)
        return np.where(has, es[idx], first_edge)

    a_e = round_edges(2 * tvec)        # vectorized: r differs per column
    b_e = round_edges(2 * tvec + 1)

    # rows 0-17: features for the w1n (zb) stream; rows 32-49: the same
    # features again for the w12 (ms-init) stream, so each superblock is a
    # single rectangular DMA and the two matmul streams read disjoint
    # partition bands (array rows 0-31 / 32-63, concurrent row groups)
    feats = np.zeros((50, S), dtype=F32)
    for half, eids in ((0, a_e), (9, b_e)):
        s, d = src[eids], dst[eids]
        feats[half + 0:half + 3, :ncols] = x[s].T
        feats[half + 3:half + 6, :ncols] = pos[s].T
        feats[half + 6:half + 9, :ncols] = pos[d].T
    feats[32:50] = feats[0:18]

    xnode = np.zeros((3, ncw), dtype=F32)
    xnode[:, :NCN] = x[core["order"] + 0].T      # caller adds core offset
    return feats, xnode


# column layouts of the packed weight tensors (bf16 matmul operands; PE runs
# fp32 at 1/4 rate, bf16 streams 1 col/cycle with f32 PSUM accumulation).
# w12 lives at partitions 32-49 so its matmuls run in array rows 32-63,
# concurrent with the w1n (rows 0-31) matmuls.
WSLOTS = dict(w1n=(0, 18, 0, 128), w12=(32, 50, 128, 128),
              w2p=(0, 128, 256, 128), g1n=(0, 67, 384, 64),
              g12=(0, 67, 448, 64), g2=(0, 64, 512, 64))
WCOL = 576
BSLOTS = dict(nbias1=(128, 0, 1), cbias=(64, 1, 1), ngb1=(64, 2, 1),
              pgb1=(64, 3, 1), gbias=(64, 4, 1), gbm1=(64, 5, 1))
BCOL = 8


def _weights(f_w1, f_b1, f_w2, f_b2, g_w1, g_b1, g_w2, g_b2):
    w9 = np.concatenate([f_w1[0:3], f_w1[3:6], -f_w1[3:6]], axis=0)  # [9,64]
    blk = lambda m: np.block([[m, np.zeros_like(m)], [np.zeros_like(m), m]])
    cbias = (f_b1 @ f_w2 - f_w2.sum(axis=0) + f_b2).astype(F32)       # [64]
    gbias = (g_b1 @ g_w2 - g_w2.sum(axis=0) + g_b2).astype(F32)       # [64]
    w = dict(
        w1n=blk(w9).astype(F32),             # [18,128]  (zb = +z1)
        w12=blk(w9 @ f_w2).astype(F32),      # [18,128]
        w2p=blk(f_w2).astype(F32),           # [128,128]
        g1n=g_w1.astype(F32),                # [67,64]
        g12=(g_w1 @ g_w2).astype(F32),       # [67,64]
        g2=g_w2.astype(F32),                 # [64,64]
        nbias1=np.tile(-f_b1, 2).astype(F32).reshape(128, 1),
        cbias=cbias.reshape(64, 1),
        ngb1=(-g_b1).astype(F32).reshape(64, 1),
        pgb1=g_b1.astype(F32).reshape(64, 1),
        gbias=gbias.reshape(64, 1),
        gbm1=(gbias - 1.0).reshape(64, 1),
    )
    import ml_dtypes
    wpack = np.zeros((128, WCOL), dtype=ml_dtypes.bfloat16)
    for name, (p0, p1, c0, cn) in WSLOTS.items():
        wpack[p0:p1, c0:c0 + cn] = w[name]
    bpack = np.zeros((128, BCOL), dtype=F32)
    for name, (p, c0, cn) in BSLOTS.items():
        bpack[:p, c0:c0 + cn] = w[name]
    w["wpack"] = wpack
    w["bpack"] = bpack
    return w


def _bf(v):
    import ml_dtypes
    return np.asarray(v).astype(ml_dtypes.bfloat16).astype(F32)


# --------------------------------------------------------------------------
# numpy model of the device program (for validation; mimics bf16 rounding)
# --------------------------------------------------------------------------

def _numpy_device(feats, xnode, w, tiles, ncw):
    G = len(tiles) // 2
    aggr = np.zeros((128, ncw), dtype=F32)
    for g in range(G):
        f = _bf(feats[0:18, g * GRP:(g + 1) * GRP])
        zb = w["w1n"].T @ f                                  # +z1
        r = _bf(np.maximum(-zb + w["nbias1"], 0))
        e = _bf(np.exp(-r))
        s = _bf(r + e)
        ms = w["w12"].T @ f + w["w2p"].T @ s
        for j in (0, 1):
            t, k = tiles[2 * g + j]
            dst = aggr[:, k * TILE:(k + 1) * TILE]
            src = _bf(ms[:, j * TILE:(j + 1) * TILE])
            if t == 0:
                dst[:] = src
            else:
                np.maximum(dst, src, out=dst)
    a64 = np.maximum(aggr[0:64], aggr[64:128])
    u_in = np.empty((67, ncw), dtype=F32)
    u_in[0:64] = _bf(np.maximum(a64 + w["cbias"], 0))
    u_in[64:67] = _bf(xnode)
    out = np.empty((64, ncw), dtype=F32)
    for i in range(ncw // TILE):
        ui = u_in[:, i * TILE:(i + 1) * TILE]
        zg = w["g1n"].T @ ui
        rg = _bf(np.maximum(-zg + w["ngb1"], 0))
        y2 = _bf(np.exp(zg + w["pgb1"]))
        sg = _bf(np.minimum(y2, 1.0) + rg)
        o2 = w["g12"].T @ ui + w["g2"].T @ sg
        y = _bf(np.exp(o2 + w["gbias"]))
        vf2 = _bf(np.maximum(o2 + w["gbm1"], -1.0))
        out[:, i * TILE:(i + 1) * TILE] = _bf(np.minimum(y, 1.0) + vf2)
    return out        # [64, ncw] (bf16-rounded values)


# --------------------------------------------------------------------------
# bass program
# --------------------------------------------------------------------------

def _import_concourse():
    try:
        import concourse.bass  # noqa: F401
    except ImportError:
        sys.path.insert(0, "/opt/trn_rl_repo")


def _install_ntff_shim():
    """Provide antenv.axon_hooks (missing in this image) so that
    run_bass_kernel_spmd(trace=True) can capture NTFF profiles."""
    import contextlib
    import ctypes
    import types

    if "antenv.axon_hooks" in sys.modules:
        return
    so_path = "/opt/axon/libaxon_pjrt.so"
    if not os.path.exists(so_path):
        return
    lib = ctypes.CDLL(so_path)
    if not hasattr(lib, "axon_start_nrt_profile"):
        return
    lib.axon_start_nrt_profile.argtypes = [ctypes.POINTER(ctypes.c_int64),
                                           ctypes.c_size_t]
    lib.axon_start_nrt_profile.restype = ctypes.c_int64
    lib.axon_stop_nrt_profile.argtypes = [ctypes.c_char_p]
    lib.axon_stop_nrt_profile.restype = ctypes.c_int64

    @contextlib.contextmanager
    def _hook(output_dir, device_ids):
        import jax
        jax.devices()
        if device_ids:
            ids = (ctypes.c_int64 * len(device_ids))(*device_ids)
            rc = lib.axon_start_nrt_profile(ids, len(device_ids))
        else:
            rc = lib.axon_start_nrt_profile(None, 0)
        if rc != 0:
            raise RuntimeError(f"axon_start_nrt_profile rc={rc}")
        try:
            yield
        finally:
            n = lib.axon_stop_nrt_profile(str(output_dir).encode())
            print(f"ntff profile: {n} file(s) -> {output_dir}",
                  file=sys.stderr)

    mod = types.ModuleType("antenv.axon_hooks")
    mod.get_axon_ntff_profile_hook = lambda: _hook
    mod.set_axon_ntff_profile_hook = lambda h: None
    sys.modules["antenv.axon_hooks"] = mod


def _dep(from_inst, to_inst, reason):
    from concourse.tile import add_dep_helper
    a = getattr(from_inst, "ins", from_inst)
    b = getattr(to_inst, "ins", to_inst)
    add_dep_helper(a, b, reason=reason)


def _build_nc(tiles, S, ncw):
    _import_concourse()
    import concourse.bass as bass
    import concourse.tile as tile
    import concourse.tile_sem_assignment as _tsa
    from concourse import mybir

    # One DMAHW bookkeeping lane: HWDGE transfers share a FIFO proc, so
    # DMA-vs-DMA ordering (slot WAW) needs no extra sync wait.
    _tsa.NUM_HWDGE_SEMS = 1

    f32 = mybir.dt.float32
    bf16 = mybir.dt.bfloat16
    AF = mybir.ActivationFunctionType
    ALU = mybir.AluOpType
    nc = bass.Bass()

    G = len(tiles) // 2
    S_pad = ((S + SUP - 1) // SUP) * SUP
    n_sup = S_pad // SUP
    n_nt = ncw // TILE                       # node tiles

    feats_d = nc.dram_tensor("feats", [50, S_pad], bf16, kind="ExternalInput")
    xnode_d = nc.dram_tensor("xnode", [3, ncw], bf16, kind="ExternalInput")
    wpack_d = nc.dram_tensor("wpack", [128, WCOL], bf16, kind="ExternalInput")
    bpack_d = nc.dram_tensor("bpack", [128, BCOL], f32, kind="ExternalInput")
    out_d = nc.dram_tensor("out", [64, ncw], bf16, kind="ExternalOutput")

    # node-phase lead-in chunks (4 tiles each) and the edge-group after which
    # each chunk's aggr columns are final (chunk 0 = blocks 0-3 is last)
    n_ck = (n_nt + 3) // 4
    ck_last = []
    for c in range(n_ck):
        blocks = set(range(4 * c, min(4 * c + 4, n_nt)))
        last = 0
        for j, (t, k) in enumerate(tiles):
            if k in blocks:
                last = j // 2
        ck_last.append(last)

    with tile.TileContext(nc) as tc:
        with (
            tc.tile_pool(name="const", bufs=1) as cpool,
            tc.tile_pool(name="aggr", bufs=1) as apool,
            tc.tile_pool(name="feats", bufs=2) as fpool,
            tc.tile_pool(name="rm", bufs=3) as rmpool,
            tc.tile_pool(name="et", bufs=2) as etpool,
            tc.tile_pool(name="st", bufs=2) as stpool,
            tc.tile_pool(name="gwork", bufs=1) as gpool,
            tc.tile_pool(name="nrg", bufs=3) as nrgpool,
            tc.tile_pool(name="ny2", bufs=2) as ny2pool,
            tc.tile_pool(name="ny", bufs=3) as nypool,
            tc.tile_pool(name="nsg", bufs=2) as nsgpool,
            tc.tile_pool(name="nvf", bufs=2) as nvfpool,
            tc.tile_pool(name="psum_z", bufs=2, space="PSUM") as pz,
            tc.tile_pool(name="psum_m", bufs=4, space="PSUM") as pm,
        ):
            wsb = cpool.tile([128, WCOL], bf16, name="wsb")
            wdma = nc.sync.dma_start(wsb[:], wpack_d[:])
            bsb = cpool.tile([128, BCOL], f32, name="bsb")
            bdma = nc.sync.dma_start(bsb[:], bpack_d[:])
            w = {name: wsb[p0:p1, c0:c0 + cn]
                 for name, (p0, p1, c0, cn) in WSLOTS.items()}
            w.update({name: bsb[0:p, c0:c0 + cn]
                      for name, (p, c0, cn) in BSLOTS.items()})
            # ACT/DVE-side absorbers: observe the bias DMA once.
            tabs = cpool.tile([1, 8], f32, name="tabs")
            ta0 = nc.scalar.activation(tabs[0:1, 0:1], bsb[0:1, 0:1], AF.Copy)
            _dep(ta0, bdma, "ACT observes bias DMA")
            vscr = cpool.tile([1, 8], f32, name="vscr")
            tv0 = nc.vector.tensor_copy(vscr[0:1, 0:1], bsb[0:1, 0:1])
            _dep(tv0, bdma, "DVE observes bias DMA")

            aggr = apool.tile([128, ncw], bf16)
            u_in = gpool.tile([67, ncw], bf16, tag="u_in")
            ah = gpool.tile([64, ncw], bf16, tag="ah")
            out_sb = gpool.tile([64, ncw], bf16, tag="out_sb")

            # ---- feats superblock staging: rows 0-17 feed the w1n (zb)
            # stream in array rows 0-31; a second copy at partitions 32-49
            # feeds the w12 (ms-init) stream in array rows 32-63 so both
            # matmuls run concurrently in different row groups.
            sup_tiles = [None] * n_sup
            sup_dmas = [None] * n_sup
            def stage_sup(i):
                st_ = fpool.tile([50, SUP], bf16, tag="feats_sup")
                d = nc.sync.dma_start(st_[:],
                                      feats_d[:, i * SUP:(i + 1) * SUP])
                sup_tiles[i] = st_
                sup_dmas[i] = d
            for i in range(min(2, n_sup)):
                stage_sup(i)

            def fcols(g, band):
                c0 = g * GRP
                st_ = sup_tiles[c0 // SUP]
                fo = c0 % SUP
                if band == 0:
                    return st_[0:18, fo:fo + GRP]
                return st_[32:50, fo:fo + GRP]

            def emit_zb(g, off):
                """one 512-col zb matmul (array rows 0-31)."""
                fa = fcols(g, 0)
                if off == 0:
                    zbt = pz.tile([128, GRP], f32, tag="zb")
                    emit_zb.cur = zbt
                zbt = emit_zb.cur
                mm = nc.tensor.matmul(zbt[:, off:off + TILE], w["w1n"],
                                      fa[:, off:off + TILE], start=True,
                                      stop=True)
                return zbt, mm

            def emit_ms(g, off):
                """one 512-col ms-init matmul (array rows 32-63).  Each
                512-col half gets its own PSUM tile so the aggregate-max of
                half 0 can start as soon as half 0's w2 matmul stops."""
                fa = fcols(g, 1)
                mst = pm.tile([128, TILE], f32, tag="ms")
                mm = nc.tensor.matmul(mst[:], w["w12"],
                                      fa[:, off:off + TILE], start=True,
                                      stop=False)
                return mst, mm

            def emit_zbms(gz, gm, after=None):
                """interleaved concurrent pairs: zb(gz) in array rows 0-31
                overlaps ms-init(gm) in rows 32-63.  The explicit PE chain
                pins the scheduler to this order (alternating row groups so
                adjacent matmuls execute concurrently)."""
                zbt = None
                msts = []
                mms = []
                prev = after
                for off in (0, TILE):
                    if gz is not None:
                        zbt, mm = emit_zb(gz, off)
                        if prev is not None:
                            _dep(mm, prev, "pin PE order")
                        prev = mm
                        mms.append(mm)
                    if gm is not None:
                        mst, mm = emit_ms(gm, off)
                        if prev is not None:
                            _dep(mm, prev, "pin PE order")
                        prev = mm
                        mms.append(mm)
                        msts.append(mst)
                return zbt, msts, mms

            def emit_re(g, zbt, prev_e):
                """r = relu(-zb-b1) [ACT], e = exp(-r) [ACT].

                r is chained after the previous group's e: that e already
                waits on the DVE s-op releasing the rm slot r reuses (rm
                bufs=3 vs et bufs=2 alignment), so r keeps only its PE wait.
                """
                rm = rmpool.tile([128, GRP], bf16, tag="rm")
                et = etpool.tile([128, GRP], bf16, tag="et")
                ri = nc.scalar.activation(rm[:], zbt[:], AF.Relu,
                                          bias=w["nbias1"], scale=-1.0)
                if prev_e is not None:
                    _dep(ri, prev_e, "rm slot WAR covered by prev e wait")
                ei = nc.scalar.activation(et[:], rm[:], AF.Exp, scale=-1.0)
                return rm, et, ei

            def emit_s(g, rm, et, prev_agg):
                """s = r + e [DVE, bf16 2x] as two 512-col halves in separate
                PE-only tiles, so each w2 matmul waits only its own half;
                chained after aggmax_t0 so the st-slot PE-WAR is covered."""
                sts = []
                prev = prev_agg
                for off in (0, TILE):
                    st_ = stpool.tile([128, TILE], bf16, tag="st")
                    si = nc.vector.tensor_add(st_[:], rm[:, off:off + TILE],
                                              et[:, off:off + TILE])
                    if prev is not None:
                        _dep(si, prev, "pin DVE order / cover st WAR")
                    prev = si
                    sts.append(st_)
                emit_s.last = si
                return sts

            def emit_w2(g, msts, sts, after=None):
                mm = []
                for j in (0, 1):
                    mmi = nc.tensor.matmul(
                        msts[j][:], w["w2p"], sts[j][:],
                        start=False, stop=True)
                    if after is not None:
                        _dep(mmi, after, "pin PE order: w2 after pairs")
                        after = None
                    mm.append(mmi)
                emit_w2.last_msts = msts
                return mm

            def emit_aggtile(g, j, pin_after=None):
                t, k = tiles[2 * g + j]
                dst = aggr[:, k * TILE:(k + 1) * TILE]
                src = emit_w2.last_msts[j][:]
                if t == 0:
                    rmx = nc.vector.tensor_copy(dst, src)
                else:
                    rmx = nc.vector.tensor_max(dst, dst, src)
                if pin_after is not None:
                    _dep(rmx, pin_after, "pin DVE order")
                return rmx

            # ---- node-phase lead-in (per 4-tile chunk): move odd-round half
            # down, fold max, relu+cbias into u_in; emitted as soon as the
            # chunk's aggr columns are final so it hides under the edge phase
            def emit_chunk(c):
                c0 = 4 * c * TILE
                cw = min(ncw - c0, 4 * TILE)
                d = nc.sync.dma_start(ah[:, c0:c0 + cw],
                                      aggr[64:128, c0:c0 + cw])
                tvc = nc.vector.tensor_copy(vscr[0:1, 1:2], bsb[0:1, 0:1])
                _dep(tvc, d, "DVE absorbs fold DMA dep")
                fo = nc.vector.tensor_max(ah[:, c0:c0 + cw],
                                          aggr[0:64, c0:c0 + cw],
                                          ah[:, c0:c0 + cw])
                _dep(fo, tvc, "order after absorber")
                ur = nc.scalar.activation(u_in[0:64, c0:c0 + cw],
                                          ah[:, c0:c0 + cw], AF.Relu,
                                          bias=w["cbias"], scale=1.0)
                return ur

            # =========== edge phase ===========
            zb_t = {}
            ms_t = {}
            rm_t = {}
            et_t = {}
            st_t = {}
            zb_t[0], ms_t[0], _ = emit_zbms(0, 0)
            zb_t[1], _, _ = emit_zbms(1, None)
            rm_t[0], et_t[0], prev_e = emit_re(0, zb_t[0], None)
            st_t[0] = emit_s(0, rm_t[0], et_t[0], None)
            prev_w2 = None

            chunks_done = set()
            chunk_insts = {}
            for g in range(G):
                # stage the superblock that groups g+2/g+3 will read
                c3 = (g + 3) * GRP
                new_sup = None
                if g + 3 < G and c3 % SUP == 0 and c3 // SUP < n_sup \
                        and sup_tiles[c3 // SUP] is None:
                    stage_sup(c3 // SUP)
                    new_sup = sup_dmas[c3 // SUP]
                # zb(g+2) || ms(g+1) concurrent pairs, pinned after the
                # previous iteration's w2 twin (which absorbed the sup DMA)
                gz = g + 2 if g + 2 < G else None
                gm = g + 1 if g + 1 < G else None
                pair_mms = None
                if gz is not None or gm is not None:
                    zbt, msts, pair_mms = emit_zbms(gz, gm, after=prev_w2)
                    if gz is not None:
                        zb_t[gz] = zbt
                    if gm is not None:
                        ms_t[gm] = msts
                    # the wait-free second zb matmul absorbs the DMA wait of
                    # the superblock next iteration's zb will read first
                    cza = (g + 3) * GRP
                    if gz is not None and g + 3 < G and cza % SUP == 0 \
                            and sup_dmas[cza // SUP] is not None:
                        _dep(pair_mms[2], sup_dmas[cza // SUP],
                             "sup prefetch via zb twin")
                if g + 1 < G:
                    rm_t[g + 1], et_t[g + 1], prev_e = emit_re(
                        g + 1, zb_t[g + 1], prev_e)
                mm_e = emit_w2(g, ms_t.pop(g), st_t.pop(g),
                               after=pair_mms[-1] if pair_mms else None)
                prev_w2 = mm_e[1]
                agg0 = emit_aggtile(g, 0)
                if g + 1 < G:
                    st_t[g + 1] = emit_s(g + 1, rm_t[g + 1], et_t[g + 1],
                                         agg0)
                    emit_aggtile(g, 1, pin_after=emit_s.last)
                else:
                    emit_aggtile(g, 1)
                rm_t.pop(g, None)
                et_t.pop(g, None)
                # early node-phase chunks once their blocks are final
                for c in range(1, n_ck):
                    if c not in chunks_done and ck_last[c] == g:
                        chunks_done.add(c)
                        chunk_insts[c] = emit_chunk(c)

            xdma = nc.sync.dma_start(u_in[64:67, :], xnode_d[:])
            chunk_insts[0] = emit_chunk(0)
            for c in range(1, n_ck):
                if c not in chunks_done:
                    chunk_insts[c] = emit_chunk(c)

            # absorber matmuls: observe the chunk-0 relu (ACT) and the xnode
            # DMA so the first node matmuls keep a single sem wait
            kwt1 = pz.tile([64, TILE], f32, tag="zb", name="kwt1")
            kw1 = nc.tensor.matmul(kwt1[:], wsb[0:64, 0:64],
                                   wsb[0:64, 0:TILE], start=True, stop=True)
            _dep(kw1, chunk_insts[0], "observe chunk-0 relu")
            kwt2 = pz.tile([64, TILE], f32, tag="zb", name="kwt2")
            kw2 = nc.tensor.matmul(kwt2[:], wsb[0:64, 0:64],
                                   wsb[0:64, 0:TILE], start=True, stop=True)
            _dep(kw2, xdma, "absorb xnode DMA wait")

            # =========== node phase ===========
            # per tile: zg = g1n@u; rg = relu(-zg-gb1); y2 = exp(zg+gb1);
            # sg = min(y2,1)+rg; o2 = g12@u + g2@sg; y = exp(o2+gbias);
            # vf2 = max(o2+gbias-1, -1); out = min(y,1)+vf2.
            # Explicit deps keep every instruction at <=1 sync wait.
            zg_t = {}
            o2_t = {}

            def emit_nmm1(i, prev_g2sg):
                ui = u_in[:, i * TILE:(i + 1) * TILE]
                zg = pz.tile([64, TILE], f32, tag="zb")
                nc.tensor.matmul(zg[:], w["g1n"], ui, start=True, stop=True)
                o2 = pm.tile([64, TILE], f32, tag="ms")
                o2i = nc.tensor.matmul(o2[:], w["g12"], ui, start=True,
                                       stop=False)
                if prev_g2sg is not None:
                    _dep(o2i, prev_g2sg, "pm slot DVE-WAR covered by g2sg")
                return zg, o2

            def emit_nact(i, prev_y2):
                rg = nrgpool.tile([64, TILE], bf16, tag="nrg")
                y2 = ny2pool.tile([64, TILE], bf16, tag="ny2")
                rgi = nc.scalar.activation(rg[:], zg_t[i][:], AF.Relu,
                                           bias=w["ngb1"], scale=-1.0)
                if prev_y2 is not None:
                    _dep(rgi, prev_y2, "nrg slot WAR covered by prev y2")
                y2i = nc.scalar.activation(y2[:], zg_t[i][:], AF.Exp,
                                           bias=w["pgb1"], scale=1.0)
                _dep(y2i, rgi, "share zg PE wait")
                return rg, y2, y2i

            def emit_nsg(i, rg, y2, prev_vf2):
                sg = nsgpool.tile([64, TILE], bf16, tag="nsg")
                sgi = nc.vector.scalar_tensor_tensor(
                    sg[:], y2[:], 1.0, rg[:], op0=ALU.min, op1=ALU.add)
                if prev_vf2 is not None:
                    _dep(sgi, prev_vf2, "nsg slot PE-WAR covered by vf2")
                return sg

            zg_t[0], o2_t[0] = emit_nmm1(0, None)
            rg_c, y2_c, y2i_c = emit_nact(0, None)
            sg_t = {0: emit_nsg(0, rg_c, y2_c, None)}
            prev_y2i = y2i_c
            prev_vf2 = None

            for i in range(n_nt):
                o2 = o2_t.pop(i)
                mm_o2 = nc.tensor.matmul(o2[:], w["g2"], sg_t.pop(i)[:],
                                         start=False, stop=True)
                if i + 1 < n_nt:
                    zg_t[i + 1], o2_t[i + 1] = emit_nmm1(i + 1, mm_o2)
                    rg_c, y2_c, y2i_c = emit_nact(i + 1, prev_y2i)
                    prev_y2i = y2i_c
                y = nypool.tile([64, TILE], bf16, tag="ny")
                yi = nc.scalar.activation(y[:], o2[:], AF.Exp,
                                          bias=w["gbias"], scale=1.0)
                _dep(yi, prev_y2i, "ny slot DVE-WAR covered by y2 wait")
                vf2 = nvfpool.tile([64, TILE], bf16, tag="nvf")
                vf2i = nc.vector.tensor_scalar(vf2[:], o2[:], w["gbm1"], -1.0,
                                               ALU.add, ALU.max)
                nc.vector.scalar_tensor_tensor(
                    out_sb[:, i * TILE:(i + 1) * TILE], y[:], 1.0, vf2[:],
                    op0=ALU.min, op1=ALU.add)
                if i + 1 < n_nt:
                    sg_t[i + 1] = emit_nsg(i + 1, rg_c, y2_c, vf2i)
                nc.sync.dma_start(out_d[:, i * TILE:(i + 1) * TILE],
                                  out_sb[:, i * TILE:(i + 1) * TILE])

    return nc


def _set_waits(inst, kept):
    """Replace an instruction's sync waits.  inst.sync_info returns a copy,
    so rebuild fresh SyncWait/SyncInfo objects and assign them back to the
    instruction."""
    import bass_rust
    news = [bass_rust.SyncWait(sync_type=x.sync_type, id=x.id,
                               wait_mode=x.wait_mode, ant_name=x.ant_name,
                               wait_value=x.wait_value, wait_reg=x.wait_reg)
            for x in kept]
    si = inst.sync_info
    ups = [bass_rust.SyncUpdate(sync_type=u.sync_type, id=u.id,
                                ant_name=u.ant_name, update_value=u.update_value)
           if False else u for u in list(si.on_update)]
    inst.sync_info = bass_rust.SyncInfo(on_wait=news, on_update=ups)


def _prune_waits(nc):
    """ISA structs carry at most one sync wait. Drop provably-redundant
    waits Tile emitted (same-engine self-waits on strict-FIFO engines;
    DMA-vs-DMA ordering subsumed by compute waits; drain-tail waits)."""
    n1 = n2 = 0
    for b in nc.m.functions[0].blocks:
        for i in b.instructions:
            si = i.sync_info
            if si is None or not si.on_wait or len(si.on_wait) < 2:
                continue
            nm = type(i).__name__
            waits = list(si.on_wait)
            if nm == "InstDrain":
                dma_w = [x for x in waits if x.ant_name.startswith("DMAHW")]
                _set_waits(i, dma_w[-1:] if dma_w else waits[-1:])
                continue
            if nm == "InstMatmult":
                act_w = [x for x in waits
                         if x.ant_name.startswith("Activation")]
                dma_w = [x for x in waits if x.ant_name.startswith("DMAHW")]
                if act_w and dma_w and len(act_w) + len(dma_w) == len(waits):
                    # sup-boundary zb matmul: its ACT slot-WAR (r read of the
                    # zb slot two groups back) is transitively enforced by the
                    # preceding w2 matmul's DVE wait (w2 <- s <- e <- r, ACT
                    # strict FIFO), so only the feats-DMA RAW must remain.
                    n1 += len(act_w)
                    _set_waits(i, dma_w)
                    continue
            if nm == "InstDMACopy":
                kept = [x for x in waits
                        if not (x.ant_name.startswith("DMAHW") or
                                x.ant_name.startswith("DMASW"))]
                if kept and len(kept) < len(waits):
                    n2 += len(waits) - len(kept)
                    _set_waits(i, kept)
                continue
            own = str(i.engine).split(".")[-1]
            kept = [x for x in waits
                    if x.ant_name.rsplit("_", 1)[0] != own]
            if len(kept) < len(waits):
                n1 += len(waits) - len(kept)
                _set_waits(i, kept)
    return n1, n2


# --------------------------------------------------------------------------
# entry points
# --------------------------------------------------------------------------

def _prepare(x, pos, edge_index, f_w1, f_b1, f_w2, f_b2,
             g_w1, g_b1, g_w2, g_b2):
    x = np.asarray(x, F32)
    pos = np.asarray(pos, F32)
    src = np.asarray(edge_index[0]).astype(np.int64)
    dst = np.asarray(edge_index[1]).astype(np.int64)
    cores = _core_layouts(edge_index)
    tiles, S, ncw = _tile_plan(cores)
    S_pad = ((S + SUP - 1) // SUP) * SUP
    packs = []
    for c, core in enumerate(cores):
        feats, xnode = _pack_core(core, tiles, S_pad, ncw, x, pos, src, dst)
        xnode[:, :NCN] = x[core["order"] + c * NCN].T
        packs.append((feats, xnode))
    w = _weights(np.asarray(f_w1, F32), np.asarray(f_b1, F32),
                 np.asarray(f_w2, F32), np.asarray(f_b2, F32),
                 np.asarray(g_w1, F32), np.asarray(g_b1, F32),
                 np.asarray(g_w2, F32), np.asarray(g_b2, F32))
    return cores, tiles, S_pad, ncw, packs, w


def _finalize(results, cores, x, g_w1, g_b1, g_w2, g_b2):
    """results: list of [64, ncw] per core -> full [N, 64] output."""
    out = np.empty((N, 64), dtype=F32)
    for c, core in enumerate(cores):
        out[core["order"] + c * NCN] = np.asarray(
            results[c], F32)[:, :NCN].T
    empties = np.concatenate([c["empty"] for c in cores])
    if empties.size:
        def celu(v):
            return np.maximum(v, 0) + np.minimum(0, np.expm1(np.minimum(v, 0)))
        u_in = np.concatenate(
            [np.zeros((empties.size, 64), F32), x[empties]], axis=1)
        u = celu(u_in @ g_w1 + g_b1)
        out[empties] = celu(u @ g_w2 + g_b2).astype(F32)
    return out


def kernel(x, pos, edge_index, f_w1, f_b1, f_w2, f_b2,
           g_w1, g_b1, g_w2, g_b2, _debug_numpy=False, _trace=False):
    x = np.asarray(x, F32)
    pos = np.asarray(pos, F32)
    cores, tiles, S_pad, ncw, packs, w = _prepare(
        x, pos, edge_index, f_w1, f_b1, f_w2, f_b2, g_w1, g_b1, g_w2, g_b2)

    if _debug_numpy:
        results = [_numpy_device(f, xn, w, tiles, ncw) for (f, xn) in packs]
        return _finalize(results, cores, x, np.asarray(g_w1, F32),
                         np.asarray(g_b1, F32), np.asarray(g_w2, F32),
                         np.asarray(g_b2, F32))

    _import_concourse()
    run_kwargs = {}
    if _trace:
        _install_ntff_shim()
        import concourse.bass_utils as _bu
        _bu.upload_artifacts = lambda tmpdir: f"file://{tmpdir}"
        import tempfile
        trace_dir = tempfile.mkdtemp(prefix="bass_trace_")
        run_kwargs = dict(tmpdir=trace_dir)
        kernel._last_trace_dir = trace_dir
    from concourse.bass_utils import run_bass_kernel_spmd

    import ml_dtypes
    bf = ml_dtypes.bfloat16
    nc = _build_nc(tiles, S_pad, ncw)
    _prune_waits(nc)
    in_maps = [{"feats": feats.astype(bf), "xnode": xnode.astype(bf),
                "wpack": w["wpack"], "bpack": w["bpack"]}
               for (feats, xnode) in packs]
    res = run_bass_kernel_spmd(nc, in_maps, list(range(CORES)), trace=_trace,
                               **run_kwargs)
    results = [res.results[c]["out"] for c in range(CORES)]
    out = _finalize(results, cores, x, np.asarray(g_w1, F32),
                    np.asarray(g_b1, F32), np.asarray(g_w2, F32),
                    np.asarray(g_b2, F32))
    if _trace:
        kernel._last_exec_time_ns = res.exec_time_ns
        kernel._last_mean_exec_time_ns = res.mean_exec_time_ns
    return out


# revision 29
# speedup vs baseline: 1.2582x; 1.2582x over previous
"""Trainium2 Bass kernel for a GNN message-passing layer.

Reference semantics (per edge e = (src j, dst i)):
    m_in  = [x_j, pos_j - pos_i]                 # [E, 6]
    h     = celu(m_in @ f_w1 + f_b1)             # [E, 64]
    msg   = relu(h @ f_w2 + f_b2)                # [E, 64]
    aggr  = segment_max(msg, dst, N); empty -> 0 # [N, 64]
    u     = celu([aggr, x] @ g_w1 + g_b1)
    out   = celu(u @ g_w2 + g_b2)                # [N, 64]

Sharding: nodes split into 8 contiguous ranges (6250/core); each core gets the
edges whose dst is in its range, so segment-max is local.  Host does
index-only work (degree-sort, round layout, gather); device does every FLOP.

Device program (v2): celu decomposed as celu(z) = relu(-z) + exp(-relu(-z))
+ z - 1.  Per 1024-column group (2 edges stacked per column):
  zb = w9@f (PSUM), then either
    A-path: r = ACT.Relu(-zb-b1), e = ACT.Exp(-r); ms += w2@r + w2@e
    D-path: m = DVE.ts(zb+b1 min 0) (= -r), e = ACT.Exp(m); ms += (-w2)@m + w2@e
  ms also accumulates w12@f (the linear z term), then DVE tensor-max into a
  bf16 running aggregate (relu+bias deferred past the max).
The PE stream is software-pipelined depth-2 (w2-streams of group g run while
zb of g+2 and ms-init of g+1 are computed) so the tensor engine never waits
on ACT; a gap-free warmup burst un-throttles the PE HAM clock gate
(1.2 -> 2.4 GHz) at kernel start and keep-warm dummies span the node-phase
lead-in.
"""

import math
import os
import sys

import numpy as np

N = 50000
E = 1600000
CORES = 8
NCN = N // CORES            # nodes per core
TILE = 512                  # fp32 matmul moving free dim / one PSUM bank
GRP = 1024                  # group width (columns) = 2 tiles
SUP = 4096                  # feats DMA staging superblock (columns) = 4 groups
F32 = np.float32
DPAT = 3                    # every DPAT-th group takes the DVE (m) path


# --------------------------------------------------------------------------
# host-side layout (index work only)
# --------------------------------------------------------------------------

def _core_layouts(edge_index):
    """Per-core node ordering + degree-sorted CSR of local edges."""
    dst = np.asarray(edge_index[1])
    cores = []
    for c in range(CORES):
        lo, hi = c * NCN, (c + 1) * NCN
        eids = np.nonzero((dst >= lo) & (dst < hi))[0]
        ldst = (dst[eids] - lo).astype(np.int64)
        deg = np.bincount(ldst, minlength=NCN)
        order = np.argsort(-deg, kind="stable")         # node ranks
        rank = np.empty(NCN, np.int64)
        rank[order] = np.arange(NCN)
        perm = np.argsort(rank[ldst], kind="stable")
        es = eids[perm]                                  # edges sorted by rank
        deg_s = deg[order]
        row_start = np.zeros(NCN + 1, np.int64)
        np.cumsum(deg_s, out=row_start[1:])
        cores.append(dict(es=es, deg_s=deg_s, row_start=row_start,
                          order=order, empty=order[deg_s == 0] + lo))
    return cores


def _tile_plan(cores):
    """Shared (SPMD-uniform) tile plan at 512-column granularity.

    tiles: list of (pair_round t, node_block k); tile covers node ranks
    [512k, 512k+512) at rounds (2t, 2t+1).  Flat consecutive pairs of tiles
    form 1024-column groups (groups may straddle rounds; the aggregate-max
    is per-tile anyway).
    """
    rmax = max(int(c["deg_s"][0]) for c in cores)
    n_pairs = (rmax + 1) // 2
    tiles = []
    for t in range(n_pairs):
        w = max(int(np.searchsorted(-c["deg_s"], -(2 * t), side="left"))
                for c in cores)      # max over cores of #nodes with deg > 2t
        if t == 0:
            w = NCN                  # every aggr column gets initialized
        for k in range(max(1, (w + TILE - 1) // TILE)):
            tiles.append((t, k))
    if len(tiles) % 2:
        assert tiles[-1][0] > 0
        tiles.append(tiles[-1])      # dup: max is idempotent, not first-touch
    S = TILE * len(tiles)
    ncw = TILE * ((NCN + TILE - 1) // TILE)
    return tiles, S, ncw


def _pack_core(core, tiles, S, ncw, x, pos, src, dst):
    """Build one core's slot->edge assignment and gather features."""
    es, deg_s, row_start = core["es"], core["deg_s"], core["row_start"]
    ncols = len(tiles) * TILE
    nvec = np.tile(np.arange(TILE, dtype=np.int64), len(tiles))  # col in tile
    kvec = np.repeat([k for (_, k) in tiles], TILE)
    tvec = np.repeat([t for (t, _) in tiles], TILE)
    node = kvec * TILE + nvec                    # node rank targeted by column

    safe_node = np.minimum(node, NCN - 1)
    ecap = len(es) - 1
    first_edge = es[np.minimum(row_start[safe_node], ecap)]  # dup fallback
    bad = (node >= NCN) | (deg_s[safe_node] == 0)
    first_edge = np.where(bad, es[0], first_edge)

    def round_edges(r):
        has = (~bad) & (deg_s[safe_node] > r)
        idx = np.minimum(row_start[safe_node] + np.where(has, r, 0), ecap)
        return np.where(has, es[idx], first_edge)

    a_e = round_edges(2 * tvec)        # vectorized: r differs per column
    b_e = round_edges(2 * tvec + 1)

    # rows 0-17: features for the w1n (zb) stream; rows 32-49: the same
    # features again for the w12 (ms-init) stream, so each superblock is a
    # single rectangular DMA and the two matmul streams read disjoint
    # partition bands (array rows 0-31 / 32-63, concurrent row groups)
    feats = np.zeros((50, S), dtype=F32)
    for half, eids in ((0, a_e), (9, b_e)):
        s, d = src[eids], dst[eids]
        feats[half + 0:half + 3, :ncols] = x[s].T
        feats[half + 3:half + 6, :ncols] = pos[s].T
        feats[half + 6:half + 9, :ncols] = pos[d].T
    feats[32:50] = feats[0:18]

    xnode = np.zeros((3, ncw), dtype=F32)
    xnode[:, :NCN] = x[core["order"] + 0].T      # caller adds core offset
    return feats, xnode


# column layouts of the packed weight tensors (bf16 matmul operands; PE runs
# fp32 at 1/4 rate, bf16 streams 1 col/cycle with f32 PSUM accumulation).
# w12 lives at partitions 32-49 so its matmuls run in array rows 32-63,
# concurrent with the w1n (rows 0-31) matmuls.
WSLOTS = dict(w1n=(0, 18, 0, 128), w12=(32, 50, 128, 128),
              w2p=(0, 128, 256, 128), g1n=(0, 67, 384, 64),
              g12=(0, 67, 448, 64), g2=(0, 64, 512, 64))
WCOL = 576
BSLOTS = dict(nbias1=(128, 0, 1), cbias=(64, 1, 1), ngb1=(64, 2, 1),
              pgb1=(64, 3, 1), gbias=(64, 4, 1), gbm1=(64, 5, 1))
BCOL = 8


def _weights(f_w1, f_b1, f_w2, f_b2, g_w1, g_b1, g_w2, g_b2):
    w9 = np.concatenate([f_w1[0:3], f_w1[3:6], -f_w1[3:6]], axis=0)  # [9,64]
    blk = lambda m: np.block([[m, np.zeros_like(m)], [np.zeros_like(m), m]])
    cbias = (f_b1 @ f_w2 - f_w2.sum(axis=0) + f_b2).astype(F32)       # [64]
    gbias = (g_b1 @ g_w2 - g_w2.sum(axis=0) + g_b2).astype(F32)       # [64]
    w = dict(
        w1n=blk(w9).astype(F32),             # [18,128]  (zb = +z1)
        w12=blk(w9 @ f_w2).astype(F32),      # [18,128]
        w2p=blk(f_w2).astype(F32),           # [128,128]
        g1n=g_w1.astype(F32),                # [67,64]
        g12=(g_w1 @ g_w2).astype(F32),       # [67,64]
        g2=g_w2.astype(F32),                 # [64,64]
        nbias1=np.tile(-f_b1, 2).astype(F32).reshape(128, 1),
        cbias=cbias.reshape(64, 1),
        ngb1=(-g_b1).astype(F32).reshape(64, 1),
        pgb1=g_b1.astype(F32).reshape(64, 1),
        gbias=gbias.reshape(64, 1),
        gbm1=(gbias - 1.0).reshape(64, 1),
    )
    import ml_dtypes
    wpack = np.zeros((128, WCOL), dtype=ml_dtypes.bfloat16)
    for name, (p0, p1, c0, cn) in WSLOTS.items():
        wpack[p0:p1, c0:c0 + cn] = w[name]
    bpack = np.zeros((128, BCOL), dtype=F32)
    for name, (p, c0, cn) in BSLOTS.items():
        bpack[:p, c0:c0 + cn] = w[name]
    w["wpack"] = wpack
    w["bpack"] = bpack
    return w


def _bf(v):
    import ml_dtypes
    return np.asarray(v).astype(ml_dtypes.bfloat16).astype(F32)


# --------------------------------------------------------------------------
# numpy model of the device program (for validation; mimics bf16 rounding)
# --------------------------------------------------------------------------

def _numpy_device(feats, xnode, w, tiles, ncw):
    G = len(tiles) // 2
    aggr = np.zeros((128, ncw), dtype=F32)
    for g in range(G):
        f = _bf(feats[0:18, g * GRP:(g + 1) * GRP])
        zb = w["w1n"].T @ f                                  # +z1
        r = _bf(np.maximum(-zb + w["nbias1"], 0))
        e = _bf(np.exp(-r))
        s = _bf(r + e)
        ms = w["w12"].T @ f + w["w2p"].T @ s
        for j in (0, 1):
            t, k = tiles[2 * g + j]
            dst = aggr[:, k * TILE:(k + 1) * TILE]
            src = _bf(ms[:, j * TILE:(j + 1) * TILE])
            if t == 0:
                dst[:] = src
            else:
                np.maximum(dst, src, out=dst)
    a64 = np.maximum(aggr[0:64], aggr[64:128])
    u_in = np.empty((67, ncw), dtype=F32)
    u_in[0:64] = _bf(np.maximum(a64 + w["cbias"], 0))
    u_in[64:67] = _bf(xnode)
    out = np.empty((64, ncw), dtype=F32)
    for i in range(ncw // TILE):
        ui = u_in[:, i * TILE:(i + 1) * TILE]
        zg = w["g1n"].T @ ui
        rg = _bf(np.maximum(-zg + w["ngb1"], 0))
        y2 = _bf(np.exp(zg + w["pgb1"]))
        sg = _bf(np.minimum(y2, 1.0) + rg)
        o2 = w["g12"].T @ ui + w["g2"].T @ sg
        y = _bf(np.exp(o2 + w["gbias"]))
        vf2 = _bf(np.maximum(o2 + w["gbm1"], -1.0))
        out[:, i * TILE:(i + 1) * TILE] = _bf(np.minimum(y, 1.0) + vf2)
    return out        # [64, ncw] (bf16-rounded values)


# --------------------------------------------------------------------------
# bass program
# --------------------------------------------------------------------------

def _import_concourse():
    try:
        import concourse.bass  # noqa: F401
    except ImportError:
        sys.path.insert(0, "/opt/trn_rl_repo")


def _install_ntff_shim():
    """Provide antenv.axon_hooks (missing in this image) so that
    run_bass_kernel_spmd(trace=True) can capture NTFF profiles."""
    import contextlib
    import ctypes
    import types

    if "antenv.axon_hooks" in sys.modules:
        return
    so_path = "/opt/axon/libaxon_pjrt.so"
    if not os.path.exists(so_path):
        return
    lib = ctypes.CDLL(so_path)
    if not hasattr(lib, "axon_start_nrt_profile"):
        return
    lib.axon_start_nrt_profile.argtypes = [ctypes.POINTER(ctypes.c_int64),
                                           ctypes.c_size_t]
    lib.axon_start_nrt_profile.restype = ctypes.c_int64
    lib.axon_stop_nrt_profile.argtypes = [ctypes.c_char_p]
    lib.axon_stop_nrt_profile.restype = ctypes.c_int64

    @contextlib.contextmanager
    def _hook(output_dir, device_ids):
        import jax
        jax.devices()
        if device_ids:
            ids = (ctypes.c_int64 * len(device_ids))(*device_ids)
            rc = lib.axon_start_nrt_profile(ids, len(device_ids))
        else:
            rc = lib.axon_start_nrt_profile(None, 0)
        if rc != 0:
            raise RuntimeError(f"axon_start_nrt_profile rc={rc}")
        try:
            yield
        finally:
            n = lib.axon_stop_nrt_profile(str(output_dir).encode())
            print(f"ntff profile: {n} file(s) -> {output_dir}",
                  file=sys.stderr)

    mod = types.ModuleType("antenv.axon_hooks")
    mod.get_axon_ntff_profile_hook = lambda: _hook
    mod.set_axon_ntff_profile_hook = lambda h: None
    sys.modules["antenv.axon_hooks"] = mod


def _dep(from_inst, to_inst, reason):
    from concourse.tile import add_dep_helper
    a = getattr(from_inst, "ins", from_inst)
    b = getattr(to_inst, "ins", to_inst)
    add_dep_helper(a, b, reason=reason)


def _build_nc(tiles, S, ncw):
    _import_concourse()
    import concourse.bass as bass
    import concourse.tile as tile
    import concourse.tile_sem_assignment as _tsa
    from concourse import mybir

    # One DMAHW bookkeeping lane: HWDGE transfers share a FIFO proc, so
    # DMA-vs-DMA ordering (slot WAW) needs no extra sync wait.
    _tsa.NUM_HWDGE_SEMS = 1

    f32 = mybir.dt.float32
    bf16 = mybir.dt.bfloat16
    AF = mybir.ActivationFunctionType
    ALU = mybir.AluOpType
    nc = bass.Bass()

    G = len(tiles) // 2
    S_pad = ((S + SUP - 1) // SUP) * SUP
    n_sup = S_pad // SUP
    n_nt = ncw // TILE                       # node tiles

    feats_d = nc.dram_tensor("feats", [50, S_pad], bf16, kind="ExternalInput")
    xnode_d = nc.dram_tensor("xnode", [3, ncw], bf16, kind="ExternalInput")
    wpack_d = nc.dram_tensor("wpack", [128, WCOL], bf16, kind="ExternalInput")
    bpack_d = nc.dram_tensor("bpack", [128, BCOL], f32, kind="ExternalInput")
    out_d = nc.dram_tensor("out", [64, ncw], bf16, kind="ExternalOutput")

    # node-phase lead-in chunks (4 tiles each) and the edge-group after which
    # each chunk's aggr columns are final (chunk 0 = blocks 0-3 is last)
    n_ck = (n_nt + 3) // 4
    ck_last = []
    for c in range(n_ck):
        blocks = set(range(4 * c, min(4 * c + 4, n_nt)))
        last = 0
        for j, (t, k) in enumerate(tiles):
            if k in blocks:
                last = j // 2
        ck_last.append(last)

    with tile.TileContext(nc) as tc:
        with (
            tc.tile_pool(name="const", bufs=1) as cpool,
            tc.tile_pool(name="aggr", bufs=1) as apool,
            tc.tile_pool(name="feats", bufs=2) as fpool,
            tc.tile_pool(name="rm", bufs=3) as rmpool,
            tc.tile_pool(name="et", bufs=2) as etpool,
            tc.tile_pool(name="st", bufs=2) as stpool,
            tc.tile_pool(name="gwork", bufs=1) as gpool,
            tc.tile_pool(name="nrg", bufs=3) as nrgpool,
            tc.tile_pool(name="ny2", bufs=2) as ny2pool,
            tc.tile_pool(name="ny", bufs=3) as nypool,
            tc.tile_pool(name="nsg", bufs=2) as nsgpool,
            tc.tile_pool(name="nvf", bufs=2) as nvfpool,
            tc.tile_pool(name="psum_z", bufs=2, space="PSUM") as pz,
            tc.tile_pool(name="psum_m", bufs=4, space="PSUM") as pm,
        ):
            wsb = cpool.tile([128, WCOL], bf16, name="wsb")
            wdma = nc.sync.dma_start(wsb[:], wpack_d[:])
            bsb = cpool.tile([128, BCOL], f32, name="bsb")
            bdma = nc.sync.dma_start(bsb[:], bpack_d[:])
            w = {name: wsb[p0:p1, c0:c0 + cn]
                 for name, (p0, p1, c0, cn) in WSLOTS.items()}
            w.update({name: bsb[0:p, c0:c0 + cn]
                      for name, (p, c0, cn) in BSLOTS.items()})
            # ACT/DVE-side absorbers: observe the bias DMA once.
            tabs = cpool.tile([1, 8], f32, name="tabs")
            ta0 = nc.scalar.activation(tabs[0:1, 0:1], bsb[0:1, 0:1], AF.Copy)
            _dep(ta0, bdma, "ACT observes bias DMA")
            vscr = cpool.tile([1, 8], f32, name="vscr")
            tv0 = nc.vector.tensor_copy(vscr[0:1, 0:1], bsb[0:1, 0:1])
            _dep(tv0, bdma, "DVE observes bias DMA")

            aggr = apool.tile([128, ncw], bf16)
            u_in = gpool.tile([67, ncw], bf16, tag="u_in")
            ah = gpool.tile([64, ncw], bf16, tag="ah")
            out_sb = gpool.tile([64, ncw], bf16, tag="out_sb")

            # ---- feats superblock staging: rows 0-17 feed the w1n (zb)
            # stream in array rows 0-31; a second copy at partitions 32-49
            # feeds the w12 (ms-init) stream in array rows 32-63 so both
            # matmuls run concurrently in different row groups.
            sup_tiles = [None] * n_sup
            sup_dmas = [None] * n_sup
            def stage_sup(i):
                st_ = fpool.tile([50, SUP], bf16, tag="feats_sup")
                d = nc.sync.dma_start(st_[:],
                                      feats_d[:, i * SUP:(i + 1) * SUP])
                sup_tiles[i] = st_
                sup_dmas[i] = d
            for i in range(min(2, n_sup)):
                stage_sup(i)

            def fcols(g, band):
                c0 = g * GRP
                st_ = sup_tiles[c0 // SUP]
                fo = c0 % SUP
                if band == 0:
                    return st_[0:18, fo:fo + GRP]
                return st_[32:50, fo:fo + GRP]

            def emit_zb(g, off):
                """one 512-col zb matmul (array rows 0-31)."""
                fa = fcols(g, 0)
                if off == 0:
                    zbt = pz.tile([128, GRP], f32, tag="zb")
                    emit_zb.cur = zbt
                zbt = emit_zb.cur
                mm = nc.tensor.matmul(zbt[:, off:off + TILE], w["w1n"],
                                      fa[:, off:off + TILE], start=True,
                                      stop=True)
                return zbt, mm

            def emit_ms(g, off):
                """one 512-col ms-init matmul (array rows 32-63).  Each
                512-col half gets its own PSUM tile so the aggregate-max of
                half 0 can start as soon as half 0's w2 matmul stops."""
                fa = fcols(g, 1)
                mst = pm.tile([128, TILE], f32, tag="ms")
                mm = nc.tensor.matmul(mst[:], w["w12"],
                                      fa[:, off:off + TILE], start=True,
                                      stop=False)
                return mst, mm

            def emit_zbms(gz, gm, after=None):
                """interleaved concurrent pairs: zb(gz) in array rows 0-31
                overlaps ms-init(gm) in rows 32-63.  The explicit PE chain
                pins the scheduler to this order (alternating row groups so
                adjacent matmuls execute concurrently)."""
                zbt = None
                msts = []
                mms = []
                prev = after
                for off in (0, TILE):
                    if gz is not None:
                        zbt, mm = emit_zb(gz, off)
                        if prev is not None:
                            _dep(mm, prev, "pin PE order")
                        prev = mm
                        mms.append(mm)
                    if gm is not None:
                        mst, mm = emit_ms(gm, off)
                        if prev is not None:
                            _dep(mm, prev, "pin PE order")
                        prev = mm
                        mms.append(mm)
                        msts.append(mst)
                return zbt, msts, mms

            def emit_re(g, zbt, prev_e):
                """r = relu(-zb-b1) [ACT], e = exp(-r) [ACT].

                r is chained after the previous group's e: that e already
                waits on the DVE s-op releasing the rm slot r reuses (rm
                bufs=3 vs et bufs=2 alignment), so r keeps only its PE wait.
                """
                rm = rmpool.tile([128, GRP], bf16, tag="rm")
                et = etpool.tile([128, GRP], bf16, tag="et")
                ri = nc.scalar.activation(rm[:], zbt[:], AF.Relu,
                                          bias=w["nbias1"], scale=-1.0)
                if prev_e is not None:
                    _dep(ri, prev_e, "rm slot WAR covered by prev e wait")
                ei = nc.scalar.activation(et[:], rm[:], AF.Exp, scale=-1.0)
                return rm, et, ei

            def emit_s(g, rm, et, prev_agg):
                """s = r + e [DVE, bf16 2x] as two 512-col halves in separate
                PE-only tiles, so each w2 matmul waits only its own half;
                chained after aggmax_t0 so the st-slot PE-WAR is covered."""
                sts = []
                prev = prev_agg
                for off in (0, TILE):
                    st_ = stpool.tile([128, TILE], bf16, tag="st")
                    si = nc.vector.tensor_add(st_[:], rm[:, off:off + TILE],
                                              et[:, off:off + TILE])
                    if prev is not None:
                        _dep(si, prev, "pin DVE order / cover st WAR")
                    prev = si
                    sts.append(st_)
                emit_s.last = si
                return sts

            def emit_w2(g, msts, sts, after=None):
                mm = []
                for j in (0, 1):
                    mmi = nc.tensor.matmul(
                        msts[j][:], w["w2p"], sts[j][:],
                        start=False, stop=True)
                    if after is not None:
                        _dep(mmi, after, "pin PE order: w2 after pairs")
                        after = None
                    mm.append(mmi)
                emit_w2.last_msts = msts
                return mm

            def emit_aggtile(g, j, pin_after=None):
                t, k = tiles[2 * g + j]
                dst = aggr[:, k * TILE:(k + 1) * TILE]
                src = emit_w2.last_msts[j][:]
                if t == 0:
                    rmx = nc.vector.tensor_copy(dst, src)
                else:
                    rmx = nc.vector.tensor_max(dst, dst, src)
                if pin_after is not None:
                    _dep(rmx, pin_after, "pin DVE order")
                return rmx

            # ---- node-phase lead-in (per 4-tile chunk): move odd-round half
            # down, fold max, relu+cbias into u_in; emitted as soon as the
            # chunk's aggr columns are final so it hides under the edge phase
            def emit_chunk(c):
                c0 = 4 * c * TILE
                cw = min(ncw - c0, 4 * TILE)
                d = nc.sync.dma_start(ah[:, c0:c0 + cw],
                                      aggr[64:128, c0:c0 + cw])
                tvc = nc.vector.tensor_copy(vscr[0:1, 1:2], bsb[0:1, 0:1])
                _dep(tvc, d, "DVE absorbs fold DMA dep")
                fo = nc.vector.tensor_max(ah[:, c0:c0 + cw],
                                          aggr[0:64, c0:c0 + cw],
                                          ah[:, c0:c0 + cw])
                _dep(fo, tvc, "order after absorber")
                ur = nc.scalar.activation(u_in[0:64, c0:c0 + cw],
                                          ah[:, c0:c0 + cw], AF.Relu,
                                          bias=w["cbias"], scale=1.0)
                return ur

            # =========== edge phase ===========
            zb_t = {}
            ms_t = {}
            rm_t = {}
            et_t = {}
            st_t = {}
            zb_t[0], ms_t[0], _ = emit_zbms(0, 0)
            zb_t[1], _, _ = emit_zbms(1, None)
            rm_t[0], et_t[0], prev_e = emit_re(0, zb_t[0], None)
            st_t[0] = emit_s(0, rm_t[0], et_t[0], None)
            prev_w2 = None

            chunks_done = set()
            chunk_insts = {}
            for g in range(G):
                # stage the superblock that groups g+2/g+3 will read
                c3 = (g + 3) * GRP
                new_sup = None
                if g + 3 < G and c3 % SUP == 0 and c3 // SUP < n_sup \
                        and sup_tiles[c3 // SUP] is None:
                    stage_sup(c3 // SUP)
                    new_sup = sup_dmas[c3 // SUP]
                # zb(g+2) || ms(g+1) concurrent pairs, pinned after the
                # previous iteration's w2 twin (which absorbed the sup DMA)
                gz = g + 2 if g + 2 < G else None
                gm = g + 1 if g + 1 < G else None
                pair_mms = None
                if gz is not None or gm is not None:
                    zbt, msts, pair_mms = emit_zbms(gz, gm, after=prev_w2)
                    if gz is not None:
                        zb_t[gz] = zbt
                    if gm is not None:
                        ms_t[gm] = msts
                    # the wait-free second zb matmul absorbs the DMA wait of
                    # the superblock next iteration's zb will read first
                    cza = (g + 3) * GRP
                    if gz is not None and g + 3 < G and cza % SUP == 0 \
                            and sup_dmas[cza // SUP] is not None:
                        _dep(pair_mms[2], sup_dmas[cza // SUP],
                             "sup prefetch via zb twin")
                if g + 1 < G:
                    rm_t[g + 1], et_t[g + 1], prev_e = emit_re(
                        g + 1, zb_t[g + 1], prev_e)
                mm_e = emit_w2(g, ms_t.pop(g), st_t.pop(g))
                prev_w2 = mm_e[1]
                agg0 = emit_aggtile(g, 0)
                if g + 1 < G:
                    st_t[g + 1] = emit_s(g + 1, rm_t[g + 1], et_t[g + 1],
                                         agg0)
                    emit_aggtile(g, 1, pin_after=emit_s.last)
                else:
                    emit_aggtile(g, 1)
                rm_t.pop(g, None)
                et_t.pop(g, None)
                # early node-phase chunks once their blocks are final
                for c in range(1, n_ck):
                    if c not in chunks_done and ck_last[c] == g:
                        chunks_done.add(c)
                        chunk_insts[c] = emit_chunk(c)

            xdma = nc.sync.dma_start(u_in[64:67, :], xnode_d[:])
            chunk_insts[0] = emit_chunk(0)
            for c in range(1, n_ck):
                if c not in chunks_done:
                    chunk_insts[c] = emit_chunk(c)

            # absorber matmuls: observe the chunk-0 relu (ACT) and the xnode
            # DMA so the first node matmuls keep a single sem wait
            kwt1 = pz.tile([64, TILE], f32, tag="zb", name="kwt1")
            kw1 = nc.tensor.matmul(kwt1[:], wsb[0:64, 0:64],
                                   wsb[0:64, 0:TILE], start=True, stop=True)
            _dep(kw1, chunk_insts[0], "observe chunk-0 relu")
            kwt2 = pz.tile([64, TILE], f32, tag="zb", name="kwt2")
            kw2 = nc.tensor.matmul(kwt2[:], wsb[0:64, 0:64],
                                   wsb[0:64, 0:TILE], start=True, stop=True)
            _dep(kw2, xdma, "absorb xnode DMA wait")

            # =========== node phase ===========
            # per tile: zg = g1n@u; rg = relu(-zg-gb1); y2 = exp(zg+gb1);
            # sg = min(y2,1)+rg; o2 = g12@u + g2@sg; y = exp(o2+gbias);
            # vf2 = max(o2+gbias-1, -1); out = min(y,1)+vf2.
            # Explicit deps keep every instruction at <=1 sync wait.
            zg_t = {}
            o2_t = {}

            def emit_nmm1(i, prev_g2sg):
                ui = u_in[:, i * TILE:(i + 1) * TILE]
                zg = pz.tile([64, TILE], f32, tag="zb")
                nc.tensor.matmul(zg[:], w["g1n"], ui, start=True, stop=True)
                o2 = pm.tile([64, TILE], f32, tag="ms")
                o2i = nc.tensor.matmul(o2[:], w["g12"], ui, start=True,
                                       stop=False)
                if prev_g2sg is not None:
                    _dep(o2i, prev_g2sg, "pm slot DVE-WAR covered by g2sg")
                return zg, o2

            def emit_nact(i, prev_y2):
                rg = nrgpool.tile([64, TILE], bf16, tag="nrg")
                y2 = ny2pool.tile([64, TILE], bf16, tag="ny2")
                rgi = nc.scalar.activation(rg[:], zg_t[i][:], AF.Relu,
                                           bias=w["ngb1"], scale=-1.0)
                if prev_y2 is not None:
                    _dep(rgi, prev_y2, "nrg slot WAR covered by prev y2")
                y2i = nc.scalar.activation(y2[:], zg_t[i][:], AF.Exp,
                                           bias=w["pgb1"], scale=1.0)
                _dep(y2i, rgi, "share zg PE wait")
                return rg, y2, y2i

            def emit_nsg(i, rg, y2, prev_vf2):
                sg = nsgpool.tile([64, TILE], bf16, tag="nsg")
                sgi = nc.vector.scalar_tensor_tensor(
                    sg[:], y2[:], 1.0, rg[:], op0=ALU.min, op1=ALU.add)
                if prev_vf2 is not None:
                    _dep(sgi, prev_vf2, "nsg slot PE-WAR covered by vf2")
                return sg

            zg_t[0], o2_t[0] = emit_nmm1(0, None)
            rg_c, y2_c, y2i_c = emit_nact(0, None)
            sg_t = {0: emit_nsg(0, rg_c, y2_c, None)}
            prev_y2i = y2i_c
            prev_vf2 = None

            for i in range(n_nt):
                o2 = o2_t.pop(i)
                mm_o2 = nc.tensor.matmul(o2[:], w["g2"], sg_t.pop(i)[:],
                                         start=False, stop=True)
                if i + 1 < n_nt:
                    zg_t[i + 1], o2_t[i + 1] = emit_nmm1(i + 1, mm_o2)
                    rg_c, y2_c, y2i_c = emit_nact(i + 1, prev_y2i)
                    prev_y2i = y2i_c
                y = nypool.tile([64, TILE], bf16, tag="ny")
                yi = nc.scalar.activation(y[:], o2[:], AF.Exp,
                                          bias=w["gbias"], scale=1.0)
                _dep(yi, prev_y2i, "ny slot DVE-WAR covered by y2 wait")
                vf2 = nvfpool.tile([64, TILE], bf16, tag="nvf")
                vf2i = nc.vector.tensor_scalar(vf2[:], o2[:], w["gbm1"], -1.0,
                                               ALU.add, ALU.max)
                nc.vector.scalar_tensor_tensor(
                    out_sb[:, i * TILE:(i + 1) * TILE], y[:], 1.0, vf2[:],
                    op0=ALU.min, op1=ALU.add)
                if i + 1 < n_nt:
                    sg_t[i + 1] = emit_nsg(i + 1, rg_c, y2_c, vf2i)
                nc.sync.dma_start(out_d[:, i * TILE:(i + 1) * TILE],
                                  out_sb[:, i * TILE:(i + 1) * TILE])

    return nc


def _set_waits(inst, kept):
    """Replace an instruction's sync waits.  inst.sync_info returns a copy,
    so rebuild fresh SyncWait/SyncInfo objects and assign them back to the
    instruction."""
    import bass_rust
    news = [bass_rust.SyncWait(sync_type=x.sync_type, id=x.id,
                               wait_mode=x.wait_mode, ant_name=x.ant_name,
                               wait_value=x.wait_value, wait_reg=x.wait_reg)
            for x in kept]
    si = inst.sync_info
    ups = [bass_rust.SyncUpdate(sync_type=u.sync_type, id=u.id,
                                ant_name=u.ant_name, update_value=u.update_value)
           if False else u for u in list(si.on_update)]
    inst.sync_info = bass_rust.SyncInfo(on_wait=news, on_update=ups)


def _prune_waits(nc):
    """ISA structs carry at most one sync wait. Drop provably-redundant
    waits Tile emitted (same-engine self-waits on strict-FIFO engines;
    DMA-vs-DMA ordering subsumed by compute waits; drain-tail waits)."""
    n1 = n2 = 0
    for b in nc.m.functions[0].blocks:
        for i in b.instructions:
            si = i.sync_info
            if si is None or not si.on_wait or len(si.on_wait) < 2:
                continue
            nm = type(i).__name__
            waits = list(si.on_wait)
            if nm == "InstDrain":
                dma_w = [x for x in waits if x.ant_name.startswith("DMAHW")]
                _set_waits(i, dma_w[-1:] if dma_w else waits[-1:])
                continue
            if nm == "InstMatmult":
                act_w = [x for x in waits
                         if x.ant_name.startswith("Activation")]
                dma_w = [x for x in waits if x.ant_name.startswith("DMAHW")]
                if act_w and dma_w and len(act_w) + len(dma_w) == len(waits):
                    # sup-boundary zb matmul: its ACT slot-WAR (r read of the
                    # zb slot two groups back) is transitively enforced by the
                    # preceding w2 matmul's DVE wait (w2 <- s <- e <- r, ACT
                    # strict FIFO), so only the feats-DMA RAW must remain.
                    n1 += len(act_w)
                    _set_waits(i, dma_w)
                    continue
            if nm == "InstDMACopy":
                kept = [x for x in waits
                        if not (x.ant_name.startswith("DMAHW") or
                                x.ant_name.startswith("DMASW"))]
                if kept and len(kept) < len(waits):
                    n2 += len(waits) - len(kept)
                    _set_waits(i, kept)
                continue
            own = str(i.engine).split(".")[-1]
            kept = [x for x in waits
                    if x.ant_name.rsplit("_", 1)[0] != own]
            if len(kept) < len(waits):
                n1 += len(waits) - len(kept)
                _set_waits(i, kept)
    return n1, n2


# --------------------------------------------------------------------------
# entry points
# --------------------------------------------------------------------------

def _prepare(x, pos, edge_index, f_w1, f_b1, f_w2, f_b2,
             g_w1, g_b1, g_w2, g_b2):
    x = np.asarray(x, F32)
    pos = np.asarray(pos, F32)
    src = np.asarray(edge_index[0]).astype(np.int64)
    dst = np.asarray(edge_index[1]).astype(np.int64)
    cores = _core_layouts(edge_index)
    tiles, S, ncw = _tile_plan(cores)
    S_pad = ((S + SUP - 1) // SUP) * SUP
    packs = []
    for c, core in enumerate(cores):
        feats, xnode = _pack_core(core, tiles, S_pad, ncw, x, pos, src, dst)
        xnode[:, :NCN] = x[core["order"] + c * NCN].T
        packs.append((feats, xnode))
    w = _weights(np.asarray(f_w1, F32), np.asarray(f_b1, F32),
                 np.asarray(f_w2, F32), np.asarray(f_b2, F32),
                 np.asarray(g_w1, F32), np.asarray(g_b1, F32),
                 np.asarray(g_w2, F32), np.asarray(g_b2, F32))
    return cores, tiles, S_pad, ncw, packs, w


def _finalize(results, cores, x, g_w1, g_b1, g_w2, g_b2):
    """results: list of [64, ncw] per core -> full [N, 64] output."""
    out = np.empty((N, 64), dtype=F32)
    for c, core in enumerate(cores):
        out[core["order"] + c * NCN] = np.asarray(
            results[c], F32)[:, :NCN].T
    empties = np.concatenate([c["empty"] for c in cores])
    if empties.size:
        def celu(v):
            return np.maximum(v, 0) + np.minimum(0, np.expm1(np.minimum(v, 0)))
        u_in = np.concatenate(
            [np.zeros((empties.size, 64), F32), x[empties]], axis=1)
        u = celu(u_in @ g_w1 + g_b1)
        out[empties] = celu(u @ g_w2 + g_b2).astype(F32)
    return out


def kernel(x, pos, edge_index, f_w1, f_b1, f_w2, f_b2,
           g_w1, g_b1, g_w2, g_b2, _debug_numpy=False, _trace=False):
    x = np.asarray(x, F32)
    pos = np.asarray(pos, F32)
    cores, tiles, S_pad, ncw, packs, w = _prepare(
        x, pos, edge_index, f_w1, f_b1, f_w2, f_b2, g_w1, g_b1, g_w2, g_b2)

    if _debug_numpy:
        results = [_numpy_device(f, xn, w, tiles, ncw) for (f, xn) in packs]
        return _finalize(results, cores, x, np.asarray(g_w1, F32),
                         np.asarray(g_b1, F32), np.asarray(g_w2, F32),
                         np.asarray(g_b2, F32))

    _import_concourse()
    run_kwargs = {}
    if _trace:
        _install_ntff_shim()
        import concourse.bass_utils as _bu
        _bu.upload_artifacts = lambda tmpdir: f"file://{tmpdir}"
        import tempfile
        trace_dir = tempfile.mkdtemp(prefix="bass_trace_")
        run_kwargs = dict(tmpdir=trace_dir)
        kernel._last_trace_dir = trace_dir
    from concourse.bass_utils import run_bass_kernel_spmd

    import ml_dtypes
    bf = ml_dtypes.bfloat16
    nc = _build_nc(tiles, S_pad, ncw)
    _prune_waits(nc)
    in_maps = [{"feats": feats.astype(bf), "xnode": xnode.astype(bf),
                "wpack": w["wpack"], "bpack": w["bpack"]}
               for (feats, xnode) in packs]
    res = run_bass_kernel_spmd(nc, in_maps, list(range(CORES)), trace=_trace,
                               **run_kwargs)
    results = [res.results[c]["out"] for c in range(CORES)]
    out = _finalize(results, cores, x, np.asarray(g_w1, F32),
                    np.asarray(g_b1, F32), np.asarray(g_w2, F32),
                    np.asarray(g_b2, F32))
    if _trace:
        kernel._last_exec_time_ns = res.exec_time_ns
        kernel._last_mean_exec_time_ns = res.mean_exec_time_ns
    return out
